# revision 47
# baseline (speedup 1.0000x reference)
"""Int8Linear TRN2 kernel: y = x @ (W_int8 * scale)^T + bias.

Column-parallel across 8 NeuronCores: each core gets a [2048, 4096] shard
of W (as W^T, contiguous), the packed x, and its bias slice.

Device strategy per core (DMA-output-byte bound):
  - leading o-groups stream HBM->SBUF via SWDGE casting DMA (int8->bf16,
    2 B/weight SBUF write); trailing FP8_GROUPS stream host-prequantized
    e4m3 weights on the sync HWDGE queue with no cast (1 B/weight).
  - weight DMAs are packed so each descriptor is >=4 KB (chunk-pairs for
    bf16, chunk-quads for fp8) -- small descriptors measured ~20% slower
    per byte.  The first 4 chunks ride single-chunk DMAs so the PE can
    start as early as possible.
  - bf16 groups: stationary x*scale in bf16, zero-padded to M=64 (M=16
    matmuls measured ~46 ns/instr slower than M=64).
  - fp8 groups: PURE fp8 matmuls; stationary is an e4m3 hi/lo split of
    x*scale*2^6 and the weights carry 2^-6 (w in [1,128] -> [2^-6, 2],
    all e4m3-normal, so the power-of-2 rescale is lossless and the
    shifts cancel in the product -- PSUM accumulates unscaled values).
    rms rel err ~1.7e-2 at FP8_GROUPS=2 (gate 2e-2), dominated by the
    e4m3 weight rounding.
  - bias is preloaded into PSUM by DVE/ACT copies; all weight matmuls
    accumulate with start=False.
  - epilogue: fp8 groups copy+add hi/lo PSUM rows on DVE; bf16 groups a
    plain ACT copy; one merged output DMA.
"""

import os

import numpy as np

IN_F = 4096
OUT_F = 16384
NT = 16
NCORES = 8
O_PER = OUT_F // NCORES  # 2048
NCH = IN_F // 128  # 32 k-chunks
NG = O_PER // 512  # 4 o-groups
FP8_GROUPS = 2  # trailing o-groups with e4m3 weights (lossy, 1B/weight)
M_PAD = 64  # stationary columns (tokens padded with zeros)
X8_SHIFT = 6  # fp8 stationary carries x*s*2^X8_SHIFT; weights carry 2^-X8_SHIFT
N_SINGLE = 4  # leading chunks delivered as single-chunk DMAs
RAMP = []  # optional small packs while the PE ramps
PACK_BF = 2  # bf16 chunks per packed DMA (4 KB descriptors -- the sweet
PACK_F8 = 4  # spot; 16 KB descs measured 20 B/ns vs 28 B/ns at 4 KB)
# All weight DMAs ride the single SWDGE queue, interleaved in PE
# consumption order: a split across sync+gpsimd starves the in-order PE
# mid-stream because the queues get ~50/50 engine share while the bf16
# stream carries 2x the fp8 bytes.
N_HOST_BF = 0

_CACHE = {}
LAST_EXEC_NS = None


def _install_drain_patch():
    """walrus codegen only allows 1 sem-wait per SP instruction; Tile's
    kernel-tail Drain aggregates many. Split them across sync nops."""
    from concourse.tile import TileContext
    from concourse.tile_scheduler import N_PROCS
    from concourse.vector_clock import VectorClock
    from bass_rust import ScopedClock

    if getattr(TileContext, "_drain_patched", False):
        return

    def _patched(self, tick_clock, wait_clock):
        gc = tick_clock.global_clock
        ticks = [gc[p] for p in range(N_PROCS)]
        for i in range(N_PROCS):
            partial = VectorClock(
                [ticks[p] if p == i else 0 for p in range(N_PROCS)]
            )
            if all(t == 0 for t in partial):
                continue
            nop = self.nc.sync.nop(hint="tail_wait", nofuse=True)
            wait_clock.add_sem_waits(nop.ins, ScopedClock({None: partial}))
        self.nc.sync.drain()
        self.nc.all_engine_barrier()
        assert self.sems is not None
        popped = self.nc._tile_sem_poison_stack.pop()
        assert popped is self._sem_poison
        self.nc.clear_and_free_semaphores(list(self.sems.allocated().values()))
        self.nc.all_engine_barrier()

    TileContext._drain_and_barrier = _patched
    TileContext._drain_patched = True


def _split_multi_waits(nc):
    """walrus codegen allows only one sem-wait per instruction: hoist all
    but the last wait of any instruction onto same-engine NoOps before it."""
    from concourse import mybir

    cnt = 0
    for fn in nc.m.functions:
        for bb in fn.blocks:
            out = []
            for inst in bb.instructions:
                si = inst.sync_info
                if si is not None and si.on_wait and len(si.on_wait) > 1:
                    waits = list(si.on_wait)
                    for w in waits[:-1]:
                        cnt += 1
                        nop = mybir.InstNoOp(
                            name=f"{inst.name}-sw{cnt}", ins=[], outs=[]
                        )
                        nop.engine = inst.engine
                        nop.sync_info = mybir.SyncInfo(on_wait=[w], on_update=[])
                        out.append(nop)
                    si.on_wait = [waits[-1]]
                out.append(inst)
            bb.instructions[:] = out


def _dma_plan(nch, pack):
    """[(start_chunk, n_chunks), ...] covering 0..nch-1: singles, small
    ramp packs, then pack-sized packs."""
    plan = [(i, 1) for i in range(N_SINGLE)]
    i = N_SINGLE
    for k in RAMP:
        if i >= nch:
            break
        k = min(k, nch - i)
        plan.append((i, k))
        i += k
    while i < nch:
        k = min(pack, nch - i)
        plan.append((i, k))
        i += k
    return plan


def _build_nc():
    import concourse.bass as bass
    import concourse.mybir as mybir
    from concourse.tile import TileContext

    _install_drain_patch()

    nbf = NG - FP8_GROUPS  # leading bf16 o-groups
    obf = nbf * 512  # bf16 out-feature columns per chunk
    of8 = O_PER - obf  # fp8 out-feature columns per chunk

    nc = bass.Bass(trn_type="TRN2")
    xt = nc.dram_tensor(
        "xt", [128, NCH * M_PAD], mybir.dt.bfloat16, kind="ExternalInput"
    )
    x8t = None
    if FP8_GROUPS:
        x8t = nc.dram_tensor(
            "x8t", [128, NCH * M_PAD], mybir.dt.float8e4, kind="ExternalInput"
        )
    by = nc.dram_tensor("by", [NT, O_PER], mybir.dt.bfloat16, kind="ExternalInput")
    n_swdge = NCH - (N_HOST_BF if nbf else 0)
    wt = None
    wb = None
    if nbf:
        # packed: row (m*128+p) holds the chunk-group's k-rows back to back
        wt = nc.dram_tensor(
            "wt", [n_swdge * 128, obf], mybir.dt.int8, kind="ExternalInput"
        )
        if N_HOST_BF:
            wb = nc.dram_tensor(
                "wb", [N_HOST_BF * 128, obf], mybir.dt.bfloat16, kind="ExternalInput"
            )
    w8 = None
    if FP8_GROUPS:
        w8 = nc.dram_tensor("w8", [IN_F, of8], mybir.dt.float8e4, kind="ExternalInput")
    y = nc.dram_tensor("y", [NT, O_PER], mybir.dt.float32, kind="ExternalOutput")

    bf_plan = _dma_plan(n_swdge, PACK_BF) if nbf else []
    wb_plan = (
        [(n_swdge + i, min(PACK_BF, NCH - n_swdge - i)) for i in range(0, NCH - n_swdge, PACK_BF)]
        if (nbf and N_HOST_BF)
        else []
    )
    f8_plan = _dma_plan(NCH, PACK_F8) if FP8_GROUPS else []

    with TileContext(nc) as tc:
        with (
            tc.tile_pool(name="xp", bufs=1) as xp,
            tc.tile_pool(name="wp", bufs=1) as wp,
            tc.tile_pool(name="pp", bufs=1, space="PSUM") as pp,
            tc.tile_pool(name="op", bufs=1) as op,
        ):
            psums = [
                pp.tile([M_PAD, 512], mybir.dt.float32, tag=f"ps{g}", name=f"ps{g}")
                for g in range(NG)
            ]
            # inputs on the sync queue: x8 then x
            x8sb = None
            if FP8_GROUPS:
                x8sb = xp.tile(
                    [128, NCH * M_PAD], mybir.dt.float8e4, tag="x8", name="x8"
                )
                nc.sync.dma_start(out=x8sb[:], in_=x8t[:])
            xsb = xp.tile([128, NCH * M_PAD], mybir.dt.bfloat16, tag="xb", name="xb")
            nc.sync.dma_start(out=xsb[:], in_=xt[:])

            # all weight DMAs on the SWDGE queue, merged in chunk order
            # (fp8 entry before the bf16 entry at the same start chunk,
            # matching the PE's per-chunk g2,g3,g0,g1 order)
            f8tiles = {}
            bftiles = {}
            merged = sorted(
                [(s, 0, k) for s, k in f8_plan] + [(s, 1, k) for s, k in bf_plan],
                key=lambda e: (e[0], e[1]),
            )
            for idx, (start, kind, k) in enumerate(merged):
                if idx == 2:
                    # bias prefill: y = broadcast bias (bf16->fp32 casting
                    # DMA; SWDGE so queue order serializes it before the
                    # accumulating output DMAs).  Slotted after chunk 0's
                    # weight DMAs so it doesn't delay the PE start.
                    # Avoids any engine->PSUM bias preload (DVE->PE PSUM
                    # write visibility races with accumulating matmuls and
                    # makes results timing-dependent).
                    nc.gpsimd.dma_start(out=y[:, :], in_=by[:, :])
                if kind == 0:
                    t = wp.tile(
                        [128, k * of8],
                        mybir.dt.float8e4,
                        tag=f"v{start}",
                        name=f"v{start}",
                    )
                    nc.gpsimd.dma_start(
                        out=t[:], in_=w8[start * 128 : (start + k) * 128, :]
                    )
                    for c in range(k):
                        f8tiles[start + c] = (t, c * of8)
                else:
                    t = wp.tile(
                        [128, k * obf],
                        mybir.dt.bfloat16,
                        tag=f"w{start}",
                        name=f"w{start}",
                    )
                    nc.gpsimd.dma_start(
                        out=t[:], in_=wt[start * 128 : (start + k) * 128, :]
                    )
                    for c in range(k):
                        bftiles[start + c] = (t, c * obf)

            for n in range(NCH):
                # fp8 groups first: their tiles arrive earlier and the PE
                # executes its stream in order
                for g in list(range(nbf, NG)) + list(range(nbf)):
                    if g < nbf:
                        tile, base = bftiles[n]
                        off = base + g * 512
                        lhs = xsb
                    else:
                        tile, base = f8tiles[n]
                        off = base + (g - nbf) * 512
                        lhs = x8sb
                    nc.tensor.matmul(
                        psums[g][:, :],
                        lhsT=lhs[:, n * M_PAD : (n + 1) * M_PAD],
                        rhs=tile[:, off : off + 512],
                        start=(n == 0),
                        stop=(n == NCH - 1),
                    )

            osb = op.tile([NT, O_PER], mybir.dt.float32, tag="o", name="osb")
            # fp8 groups: osb = ps_lo * 2^-SHIFT, then += ps_hi (DVE);
            # bf16 groups: plain copy (ACT)
            for g in range(nbf, NG):
                sl = osb[:, g * 512 : (g + 1) * 512]
                nc.vector.tensor_scalar_mul(
                    sl, psums[g][32:48, :], float(2.0**-X8_SHIFT)
                )
                nc.vector.tensor_add(sl, sl, psums[g][0:NT, :])
            for g in range(nbf):
                nc.scalar.copy(
                    osb[:, g * 512 : (g + 1) * 512], psums[g][0:NT, :]
                )
            # accumulating output DMA on the SWDGE queue (y already holds
            # bias): y += osb
            nc.gpsimd.dma_start(
                out=y[:, :], in_=osb[:, :], accum_op=mybir.AluOpType.add
            )
    _split_multi_waits(nc)
    return nc


def _pack_chunks(arr2d, plan, cols):
    """arr2d [IN_F, cols] -> packed [IN_F, cols] where each plan entry's
    chunks are laid out back to back along the row for each partition."""
    out = np.empty_like(arr2d)
    for start, k in plan:
        blk = arr2d[start * 128 : (start + k) * 128, :]  # [k*128, cols]
        # -> [128, k*cols]: partition p gets chunks start..start+k-1
        packed = blk.reshape(k, 128, cols).transpose(1, 0, 2).reshape(128, k * cols)
        out[start * 128 : (start + k) * 128, :] = packed.reshape(
            128 * k, cols
        )  # flat rows, contiguous per partition
    return out


def kernel(x, weight_int8, weight_scale, bias):
    global LAST_EXEC_NS
    import ml_dtypes
    from concourse.bass_utils import run_bass_kernel_spmd

    x = np.asarray(x, dtype=np.float32)
    w = np.asarray(weight_int8)
    if w.dtype != np.int8:
        w = w.astype(np.int8)
    scale = float(np.asarray(weight_scale, dtype=np.float32))
    bias = np.asarray(bias, dtype=np.float32)

    nbf = NG - FP8_GROUPS
    obf = nbf * 512
    of8 = O_PER - obf

    # bf16 stationary: cols 0:NT = x*s, NT:M_PAD zero
    xs = (x * np.float32(scale)).astype(ml_dtypes.bfloat16)  # [NT, IN_F]
    xtf = np.zeros((IN_F, M_PAD), dtype=ml_dtypes.bfloat16)
    xtf[:, :NT] = xs.T
    xt_host = np.ascontiguousarray(
        xtf.reshape(NCH, 128, M_PAD).transpose(1, 0, 2).reshape(128, NCH * M_PAD)
    )

    # fp8 stationary: hi/lo split of x*s*2^X8_SHIFT
    x8_host = None
    if FP8_GROUPS:
        v = x * np.float32(scale * 2.0**X8_SHIFT)  # [NT, IN_F]
        xh = v.astype(ml_dtypes.float8_e4m3)
        # lo residual boosted by another 2^SHIFT so it stays e4m3-normal;
        # its product with the 2^-SHIFT weights lands 2^SHIFT hot and the
        # epilogue rescales it
        xl = ((v - xh.astype(np.float32)) * np.float32(2.0**X8_SHIFT)).astype(
            ml_dtypes.float8_e4m3
        )
        x8f = np.zeros((IN_F, M_PAD), dtype=ml_dtypes.float8_e4m3)
        x8f[:, :NT] = xh.T
        x8f[:, 32:48] = xl.T  # lo lands on PSUM rows 32:48 (32-aligned)
        x8_host = np.ascontiguousarray(
            x8f.reshape(NCH, 128, M_PAD).transpose(1, 0, 2).reshape(128, NCH * M_PAD)
        )

    n_swdge = NCH - (N_HOST_BF if nbf else 0)
    bf_plan = _dma_plan(n_swdge, PACK_BF) if nbf else []
    wb_plan = (
        [(i, min(PACK_BF, NCH - i)) for i in range(n_swdge, NCH, PACK_BF)]
        if (nbf and N_HOST_BF)
        else []
    )
    f8_plan = _dma_plan(NCH, PACK_F8) if FP8_GROUPS else []

    if "nc" not in _CACHE:
        _CACHE["nc"] = _build_nc()
    nc = _CACHE["nc"]

    in_maps = []
    for c in range(NCORES):
        wshard = w[c * O_PER : (c + 1) * O_PER, :]  # [2048, 4096]
        wt_c = np.ascontiguousarray(wshard.T)  # [4096, 2048] int8
        bshard = bias[c * O_PER : (c + 1) * O_PER]
        byv = np.ascontiguousarray(
            np.broadcast_to(
                bshard.astype(ml_dtypes.bfloat16)[None, :], (NT, O_PER)
            )
        )
        m = {"xt": xt_host, "by": byv}
        if FP8_GROUPS:
            m["x8t"] = x8_host
        if nbf:
            wcols = np.ascontiguousarray(wt_c[:, :obf])
            m["wt"] = _pack_chunks(wcols[: n_swdge * 128], bf_plan, obf)
            if N_HOST_BF:
                wbf = wcols[n_swdge * 128 :].astype(ml_dtypes.bfloat16)
                m["wb"] = _pack_chunks(
                    wbf, [(s - n_swdge, k) for s, k in wb_plan], obf
                )
        if FP8_GROUPS:
            # 2^-X8_SHIFT keeps |w| in [2^-6, 2]: all e4m3-normal, same
            # relative grid as the raw ints (lossless rescale)
            w8full = (
                wt_c[:, obf:].astype(np.float32) * np.float32(2.0**-X8_SHIFT)
            ).astype(ml_dtypes.float8_e4m3)
            m["w8"] = _pack_chunks(np.ascontiguousarray(w8full), f8_plan, of8)
        in_maps.append(m)

    trace = bool(os.environ.get("BASS_KERNEL_TRACE"))
    br = run_bass_kernel_spmd(nc, in_maps, list(range(NCORES)), trace=trace)
    LAST_EXEC_NS = br.exec_time_ns
    return np.concatenate([br.results[c]["y"] for c in range(NCORES)], axis=1)
